# revision 19
# baseline (speedup 1.0000x reference)
"""Causal self-attention + residual + LayerNorm fused Trainium2 kernel (v2).

Problem: B=4, S=2048, D=1024, H=16 heads (hd=64), fp32 in/out.
    qkv = x @ in_proj_w.T + in_proj_b ; causal MHA ; out proj ; y = LN(x + attn_out)

Sharding (zero cross-core communication, 8 NeuronCores):
    core c -> batch b = c % 4, query-group g = c // 4.
    Causal zig-zag balance: g=0 owns query blocks [0:512) and [1536:2048),
    g=1 owns [512:1536). Every core computes full K/V for its batch,
    attention only for its own 1024 queries, then out-proj + residual +
    LayerNorm for its queries. Outputs are disjoint row sets.

v2 changes vs v1 (820us):
  - All matmul operands bf16 (same PE rate as f32r at free>=256, but FWL
    halves LDWEIGHTS and DMA bytes halve; DVE gets 2x on 16-bit ops).
  - Projections restructured so one LDWEIGHTS feeds 2-4 matmuls.
  - Whole per-core program lives inside one tc.If branch so the Tile
    scheduler can interleave projections / attention / LN freely -> PE
    stays dense enough to keep the HAM clock gate at 2.4 GHz (v1 spent
    546us of 821us throttled at 1.2 GHz).
  - Causal diagonal band (last 4 k-tiles of each q-tile) restricts the
    matmul/exp/mask free dim to the valid query suffix; mask multiply
    shrinks to one [128,2x128] bf16 op on the diagonal block only.
  - exp for both heads of a pair merged into one [128,2,F] ACTIVATE from
    a 2-bank PSUM tile.
  - softmax denominator: V is augmented with a ones column (PSUM row 64),
    reciprocal_approx_fast on [1,2,512] per head-pair, partition-broadcast
    on GpSimd, normalize on DVE.
  - LayerNorm rstd via exp(-0.5*ln(var+eps)) -- keeps the single Act
    table (exp/identity/copy/square/ln) loaded, no table swaps.
  - V projection bias folded into the out-proj bias on the host
    (softmax rows sum to 1): bo' = out_b + out_w @ bv.
"""
import sys

if "/opt/trn_rl_repo" not in sys.path:
    sys.path.insert(0, "/opt/trn_rl_repo")

import numpy as np

B, S, D, H, HD = 4, 2048, 1024, 16, 64
P = 128
QT = 512                      # queries per q-tile
NQ = 1024                     # queries per core
NKT = S // P                  # 16 k-tiles per batch
DK = D // P                   # 8 contraction tiles over D
QSTART = {0: (0, 3 * QT), 1: (QT, 2 * QT)}   # group -> per-q-tile query start
NKS = {0: (4, 16), 1: (8, 12)}               # group -> per-q-tile k-tile count

_cache = {}


def _build():
    import concourse.mybir as mybir
    import concourse.tile as tile
    from concourse import bacc
    from concourse.bass import ts
    from concourse.alu_op_type import AluOpType

    f32 = mybir.dt.float32
    f32r = mybir.dt.float32r
    bf16 = mybir.dt.bfloat16
    AF = mybir.ActivationFunctionType

    nc = bacc.Bacc("TRN2", target_bir_lowering=False, debug=False, num_devices=8)

    xq32 = nc.dram_tensor("xq32", [D, NQ], f32, kind="ExternalInput").ap()
    xtb = nc.dram_tensor("xtb", [D, S], bf16, kind="ExternalInput").ap()
    wqkv = nc.dram_tensor("wqkv", [D, 3 * D], bf16, kind="ExternalInput").ap()
    wotd = nc.dram_tensor("wotd", [D, D], bf16, kind="ExternalInput").ap()
    mskd = nc.dram_tensor("mskd", [P, 2 * P], bf16, kind="ExternalInput").ap()
    bqd = nc.dram_tensor("bqd", [D], f32, kind="ExternalInput").ap()
    bkd = nc.dram_tensor("bkd", [D], f32, kind="ExternalInput").ap()
    bod = nc.dram_tensor("bod", [D], f32, kind="ExternalInput").ap()
    gamd = nc.dram_tensor("gamd", [D], f32, kind="ExternalInput").ap()
    betd = nc.dram_tensor("betd", [D], f32, kind="ExternalInput").ap()
    yt = nc.dram_tensor("yt", [D, NQ], f32, kind="ExternalOutput").ap()

    xt_r = xtb.rearrange("(dk p) t -> p dk t", p=P)
    xq32_r = xq32.rearrange("(ok p) q -> p ok q", p=P)
    w_r = wqkv.rearrange("(dk p) (s c) -> p dk s c", p=P, c=P)  # 24 sections
    wot_r = wotd.rearrange("(dk p) (o c) -> p dk o c", p=P, c=P)

    with tile.TileContext(nc) as tc:
        with tc.tile_pool(name="pers", bufs=1) as pers:
            msk = pers.tile([P, 2, P], bf16)
            bias = pers.tile([P, DK, 5], f32)      # bq bk bo' gam bet
            ones = pers.tile([P, 1], f32r)
            eps = pers.tile([1, 1], f32)

            def bq_(f): return bias[:, f, 0:1]
            def bk_(f): return bias[:, f, 1:2]
            def bo_(f): return bias[:, f, 2:3]
            def gam_(f): return bias[:, f, 3:4]
            def bet_(f): return bias[:, f, 4:5]

            def body(g):
                qss, nks = QSTART[g], NKS[g]
                with tc.tile_pool(name="big", bufs=1) as big:
                    kt = big.tile([P, DK, S], bf16, name=f"kt{g}")
                    v = big.tile([P, NKT, H, HD + 1], bf16, name=f"v{g}")
                    qa = big.tile([P, DK, NQ], bf16, name=f"qa{g}")
                    ctxt0 = big.tile([P, DK, QT], bf16, name=f"ctxt{g}_0")
                    ctxt1 = big.tile([P, DK, QT], bf16, name=f"ctxt{g}_1")
                    y = big.tile([P, DK, QT], f32r, name=f"y{g}")
                    nc.vector.memset(v[:, :, :, HD], 1.0)
                    nc.vector.memset(eps[:], 1e-5)
                    nc.vector.memset(ones[:].bitcast(f32), 1.0 / D)

                    # ---- projections: K, V, Q -------------------------
                    with (
                        tc.tile_pool(name="xtp", bufs=1) as xtp,
                        tc.tile_pool(name="wsp", bufs=2) as wsp,
                        tc.tile_pool(name="pp8", bufs=4, space="PSUM") as pp8,
                    ):
                        xt = xtp.tile([P, DK, S], bf16, name=f"xt{g}")
                        # K^T: stationary w chunk reused over 4 t-tiles
                        wk = wsp.tile([P, DK, 8, P], bf16, tag="w", name=f"wk{g}")
                        nc.gpsimd.dma_start(wk[:], w_r[:, :, 8:16, :])
                        for t in range(4):
                            nc.sync.dma_start(
                                xt[:, :, ts(t, QT)], xt_r[:, :, ts(t, QT)])
                        for j, bsrc in enumerate((bqd, bkd, bod, gamd, betd)):
                            nc.sync.dma_start(
                                bias[:, :, j], bsrc.rearrange("(f p) -> p f", p=P))
                        nc.sync.dma_start(
                            msk[:], mskd.rearrange("p (h c) -> p h c", c=P))
                        for f in range(DK):
                            pts = [
                                pp8.tile([P, QT], f32, tag="pp", name=f"pk{g}_{f}_{t}")
                                for t in range(4)
                            ]
                            for dk in range(DK):
                                for t in range(4):
                                    nc.tensor.matmul(
                                        pts[t][:], wk[:, dk, f, :],
                                        xt[:, dk, ts(t, QT)],
                                        start=(dk == 0), stop=(dk == DK - 1),
                                    )
                            for t in range(4):
                                nc.scalar.add(kt[:, f, ts(t, QT)], pts[t][:], bk_(f))

                        # V natural: stationary x chunk reused over 2 f-groups
                        wv = wsp.tile([P, DK, 8, P], bf16, tag="w", name=f"wv{g}")
                        nc.gpsimd.dma_start(wv[:], w_r[:, :, 16:24, :])
                        for t in range(NKT):
                            pv = [
                                pp8.tile([P, 8, HD], f32, tag="pp", name=f"pv{g}_{t}_{fg}")
                                for fg in range(2)
                            ]
                            for dk in range(DK):
                                for fg in range(2):
                                    nc.tensor.matmul(
                                        pv[fg][:], xt[:, dk, ts(t, P)],
                                        wv[:, dk, 4 * fg:4 * (fg + 1), :],
                                        start=(dk == 0), stop=(dk == DK - 1),
                                    )
                            for fg in range(2):
                                nc.scalar.copy(
                                    v[:, t, 8 * fg:8 * (fg + 1), 0:HD], pv[fg][:])

                        # Q^T: stationary w chunk reused over both q-tiles
                        wq = wsp.tile([P, DK, 8, P], bf16, tag="w", name=f"wq{g}")
                        nc.gpsimd.dma_start(wq[:], w_r[:, :, 0:8, :])
                        for f in range(DK):
                            pq = [
                                pp8.tile([P, QT], f32, tag="pp", name=f"pq{g}_{f}_{qt}")
                                for qt in range(2)
                            ]
                            for dk in range(DK):
                                for qt in range(2):
                                    nc.tensor.matmul(
                                        pq[qt][:], wq[:, dk, f, :],
                                        xt[:, dk, qss[qt]:qss[qt] + QT],
                                        start=(dk == 0), stop=(dk == DK - 1),
                                    )
                            for qt in range(2):
                                nc.scalar.add(qa[:, f, ts(qt, QT)], pq[qt][:], bq_(f))

                    # ---- attention + out-proj + LN --------------------
                    with (
                        tc.tile_pool(name="sps", bufs=2, space="PSUM") as sps,
                        tc.tile_pool(name="cps", bufs=2, space="PSUM") as cps,
                        tc.tile_pool(name="pp2", bufs=2, space="PSUM") as pp2,
                        tc.tile_pool(name="sep", bufs=3) as sep,
                        tc.tile_pool(name="wop", bufs=2) as wop,
                        tc.tile_pool(name="ep", bufs=2) as ep,
                    ):
                        for qt in range(2):
                            nk = nks[qt]
                            qb = qt * QT
                            ctx_t = ctxt0 if qt == 0 else ctxt1
                            # ---- attention ----
                            for hp in range(H // 2):
                                cp0 = cps.tile([HD + 1, QT], f32, tag="cp",
                                               name=f"cp0_{g}_{qt}_{hp}")
                                cp1 = cps.tile([HD + 1, QT], f32, tag="cp",
                                               name=f"cp1_{g}_{qt}_{hp}")
                                for i in range(nk):
                                    bi = i - (nk - 4)
                                    off = P * bi if bi > 0 else 0
                                    sp = sps.tile([P, 2, QT], f32, tag="sp",
                                                  name=f"sp{g}_{qt}_{hp}_{i}")
                                    nc.tensor.matmul(
                                        sp[:, 0, off:], kt[0:HD, hp, ts(i, P)],
                                        qa[0:HD, hp, qb + off:qb + QT],
                                        start=True, stop=True,
                                    )
                                    nc.tensor.matmul(
                                        sp[:, 1, off:], kt[HD:P, hp, ts(i, P)],
                                        qa[HD:P, hp, qb + off:qb + QT],
                                        start=True, stop=True,
                                    )
                                    se = sep.tile([P, 2, QT], bf16, tag="se",
                                                  name=f"se{g}_{qt}_{hp}_{i}")
                                    nc.scalar.activation(
                                        se[:, :, off:], sp[:, :, off:],
                                        AF.Exp, scale=0.125)
                                    if bi >= 0:
                                        nc.vector.tensor_mul(
                                            se[:, :, off:off + P],
                                            se[:, :, off:off + P], msk[:])
                                    nc.tensor.matmul(
                                        cp0[:, off:], v[:, i, 2 * hp, :],
                                        se[:, 0, off:],
                                        start=(i == 0), stop=(i == nk - 1),
                                    )
                                    nc.tensor.matmul(
                                        cp1[:, off:], v[:, i, 2 * hp + 1, :],
                                        se[:, 1, off:],
                                        start=(i == 0), stop=(i == nk - 1),
                                    )
                                den = ep.tile([1, 2, QT], f32, tag="den", bufs=1,
                                              name=f"den{g}_{qt}_{hp}")
                                dcp = nc.scalar.copy if qt == 0 else (
                                    lambda o_, i_: nc.vector.tensor_copy(o_, i_))
                                dcp(den[:, 0, :], cp0[HD:HD + 1, :])
                                dcp(den[:, 1, :], cp1[HD:HD + 1, :])
                                rden = ep.tile([1, 2 * QT], f32, tag="rden",
                                               name=f"rden{g}_{qt}_{hp}")
                                for j in range(2):
                                    nc.vector.reciprocal_approx_fast(
                                        rden[:, ts(j, QT)], den[:, j, :])
                                bch = ep.tile([HD, 2 * QT], f32, tag="bch",
                                              name=f"bch{g}_{qt}_{hp}")
                                nc.gpsimd.partition_broadcast(bch[:], rden[:])
                                nc.vector.tensor_mul(
                                    ctx_t[0:HD, hp, :], cp0[0:HD, :],
                                    bch[:, ts(0, QT)])
                                nc.vector.tensor_mul(
                                    ctx_t[HD:P, hp, :], cp1[0:HD, :],
                                    bch[:, ts(1, QT)])

                        for qt in range(2):
                            ctx_t = ctxt0 if qt == 0 else ctxt1
                            # ---- out proj + residual + LN ----
                            for o in range(DK):
                                wo = wop.tile([P, DK, P], bf16, tag="wo",
                                              name=f"wo{g}_{qt}_{o}")
                                nc.gpsimd.dma_start(wo[:], wot_r[:, :, o, :])
                                ps = pp2.tile([P, QT], f32, tag="pp2",
                                              name=f"po{g}_{qt}_{o}")
                                for dk in range(DK):
                                    nc.tensor.matmul(
                                        ps[:], wo[:, dk, :], ctx_t[:, dk, :],
                                        start=(dk == 0), stop=(dk == DK - 1),
                                    )
                                xr = ep.tile([P, QT], f32, tag="xr", bufs=3,
                                             name=f"xr{g}_{qt}_{o}")
                                nc.sync.dma_start(xr[:], xq32_r[:, o, ts(qt, QT)])
                                nc.vector.scalar_tensor_tensor(
                                    y[:, o, :], ps[:], bo_(o), xr[:],
                                    AluOpType.add, AluOpType.add,
                                )
                            mu_ps = pp2.tile([P, QT], f32, tag="pp2",
                                             name=f"mups{g}_{qt}")
                            for o in range(DK):
                                nc.tensor.matmul(
                                    mu_ps[0:1, :], ones[:], y[:, o, :],
                                    start=(o == 0), stop=(o == DK - 1))
                            ysq = ep.tile([P, DK, QT], f32r, tag="ynq", bufs=1,
                                          name=f"ysq{g}_{qt}")
                            ms_ps = pp2.tile([P, QT], f32, tag="pp2",
                                             name=f"msps{g}_{qt}")
                            for o in range(DK):
                                if qt == 0:
                                    nc.vector.tensor_mul(
                                        ysq[:, o, :], y[:, o, :], y[:, o, :])
                                else:
                                    nc.scalar.square(
                                        ysq[:, o, :], y[:, o, :].bitcast(f32))
                                nc.tensor.matmul(
                                    ms_ps[0:1, :], ones[:], ysq[:, o, :],
                                    start=(o == 0), stop=(o == DK - 1))
                            mu = ep.tile([1, QT], f32, tag="mu", bufs=1, name=f"mu{g}_{qt}")
                            nc.vector.tensor_copy(mu[:], mu_ps[0:1, :])
                            musq = ep.tile([1, QT], f32, tag="musq", bufs=1,
                                           name=f"musq{g}_{qt}")
                            nc.vector.tensor_mul(musq[:], mu[:], mu[:])
                            var = ep.tile([1, QT], f32, tag="var", bufs=1, name=f"var{g}_{qt}")
                            nc.vector.tensor_sub(var[:], ms_ps[0:1, :], musq[:])
                            lnv = ep.tile([1, QT], f32, tag="lnv", bufs=1, name=f"lnv{g}_{qt}")
                            nc.scalar.activation(lnv[:], var[:], AF.Ln, bias=eps[:])
                            rstd = ep.tile([1, QT], f32, tag="rstd",
                                           name=f"rstd{g}_{qt}")
                            nc.scalar.activation(rstd[:], lnv[:], AF.Exp, scale=-0.5)
                            mu_bc = ep.tile([P, QT], f32, tag="mu_bc", bufs=1,
                                            name=f"mubc{g}_{qt}")
                            nc.gpsimd.partition_broadcast(mu_bc[:], mu[:])
                            rs_bc = ep.tile([P, QT], f32, tag="rs_bc", bufs=1,
                                            name=f"rsbc{g}_{qt}")
                            nc.gpsimd.partition_broadcast(rs_bc[:], rstd[:])
                            mu_b = mu_bc[:].unsqueeze(1).broadcast_to([P, DK, QT])
                            rs_b = rs_bc[:].unsqueeze(1).broadcast_to([P, DK, QT])
                            yn = ep.tile([P, DK, QT], f32, tag="ynq", bufs=1,
                                         name=f"yn{g}_{qt}")
                            nc.vector.tensor_sub(yn[:], y[:].bitcast(f32), mu_b)
                            nc.vector.tensor_mul(yn[:], yn[:], rs_b)
                            for o in range(DK):
                                yo = ep.tile([P, QT], f32, tag="yo",
                                             name=f"yo{g}_{qt}_{o}")
                                nc.vector.tensor_scalar(
                                    yo[:], yn[:, o, :], gam_(o), bet_(o),
                                    AluOpType.mult, AluOpType.add,
                                )
                                nc.sync.dma_start(yt[ts(o, P), ts(qt, QT)], yo[:])

            pid = nc.partition_id()
            with tc.If(pid < 4) as cmp:
                body(0)
            with cmp.Else():
                body(1)
    nc.compile()
    return nc


def _get_nc():
    if "nc" not in _cache:
        _cache["nc"] = _build()
    return _cache["nc"]


def _prep(x, in_proj_w, in_proj_b, out_w, out_b, gamma, beta):
    import ml_dtypes
    bf16 = ml_dtypes.bfloat16

    x = np.asarray(x, np.float32)
    w = np.asarray(in_proj_w, np.float32)
    wt = np.ascontiguousarray(w.T)                          # [D, 3D]
    wo = np.asarray(out_w, np.float32)
    wot = np.ascontiguousarray(wo.T)                        # [D, D]
    bqkv = np.asarray(in_proj_b, np.float32)
    bo2 = np.asarray(out_b, np.float32) + wo @ bqkv[2 * D:3 * D]
    gam = np.asarray(gamma, np.float32)
    bet = np.asarray(beta, np.float32)
    tri = (np.arange(P)[:, None] <= np.arange(P)[None, :])
    mskd = np.ascontiguousarray(
        np.concatenate([tri, tri], axis=1).astype(bf16))
    wqkv_b = wt.astype(bf16)
    wot_b = wot.astype(bf16)
    qcols = {
        0: np.r_[0:QT, 3 * QT:4 * QT],
        1: np.r_[QT:3 * QT],
    }
    in_maps = []
    for c in range(8):
        b, g = c % 4, c // 4
        xt32 = np.ascontiguousarray(x[b].T)
        in_maps.append({
            "xtb": xt32.astype(bf16),
            "xq32": np.ascontiguousarray(xt32[:, qcols[g]]),
            "wqkv": wqkv_b,
            "wotd": wot_b,
            "mskd": mskd,
            "bqd": bqkv[0:D], "bkd": bqkv[D:2 * D], "bod": bo2,
            "gamd": gam, "betd": bet,
        })
    return in_maps, qcols


def _run(in_maps, trace=False, **kw):
    from concourse.bass_utils import run_bass_kernel_spmd

    return run_bass_kernel_spmd(_get_nc(), in_maps, list(range(8)), trace=trace, **kw)


def kernel(x, in_proj_w, in_proj_b, out_w, out_b, gamma, beta):
    in_maps, qcols = _prep(x, in_proj_w, in_proj_b, out_w, out_b, gamma, beta)
    res = _run(in_maps)
    out = np.empty((B, S, D), np.float32)
    for c in range(8):
        out[c % 4, qcols[c // 4]] = res.results[c]["yt"].T
    return out


# revision 23
# speedup vs baseline: 1.0463x; 1.0463x over previous
"""Causal self-attention + residual + LayerNorm fused Trainium2 kernel (v2).

Problem: B=4, S=2048, D=1024, H=16 heads (hd=64), fp32 in/out.
    qkv = x @ in_proj_w.T + in_proj_b ; causal MHA ; out proj ; y = LN(x + attn_out)

Sharding (zero cross-core communication, 8 NeuronCores):
    core c -> batch b = c % 4, query-group g = c // 4.
    Causal zig-zag balance: g=0 owns query blocks [0:512) and [1536:2048),
    g=1 owns [512:1536). Every core computes full K/V for its batch,
    attention only for its own 1024 queries, then out-proj + residual +
    LayerNorm for its queries. Outputs are disjoint row sets.

v2 changes vs v1 (820us):
  - All matmul operands bf16 (same PE rate as f32r at free>=256, but FWL
    halves LDWEIGHTS and DMA bytes halve; DVE gets 2x on 16-bit ops).
  - Projections restructured so one LDWEIGHTS feeds 2-4 matmuls.
  - Whole per-core program lives inside one tc.If branch so the Tile
    scheduler can interleave projections / attention / LN freely -> PE
    stays dense enough to keep the HAM clock gate at 2.4 GHz (v1 spent
    546us of 821us throttled at 1.2 GHz).
  - Causal diagonal band (last 4 k-tiles of each q-tile) restricts the
    matmul/exp/mask free dim to the valid query suffix; mask multiply
    shrinks to one [128,2x128] bf16 op on the diagonal block only.
  - exp for both heads of a pair merged into one [128,2,F] ACTIVATE from
    a 2-bank PSUM tile.
  - softmax denominator: V is augmented with a ones column (PSUM row 64),
    reciprocal_approx_fast on [1,2,512] per head-pair, partition-broadcast
    on GpSimd, normalize on DVE.
  - LayerNorm rstd via exp(-0.5*ln(var+eps)) -- keeps the single Act
    table (exp/identity/copy/square/ln) loaded, no table swaps.
  - V projection bias folded into the out-proj bias on the host
    (softmax rows sum to 1): bo' = out_b + out_w @ bv.
"""
import sys

if "/opt/trn_rl_repo" not in sys.path:
    sys.path.insert(0, "/opt/trn_rl_repo")

import numpy as np

B, S, D, H, HD = 4, 2048, 1024, 16, 64
P = 128
QT = 512                      # queries per q-tile
NQ = 1024                     # queries per core
NKT = S // P                  # 16 k-tiles per batch
DK = D // P                   # 8 contraction tiles over D
QSTART = {0: (0, 3 * QT), 1: (QT, 2 * QT)}   # group -> per-q-tile query start
NKS = {0: (4, 16), 1: (8, 12)}               # group -> per-q-tile k-tile count

_cache = {}


def _build():
    import concourse.mybir as mybir
    import concourse.tile as tile
    from concourse import bacc
    from concourse.bass import ts
    from concourse.alu_op_type import AluOpType

    f32 = mybir.dt.float32
    f32r = mybir.dt.float32r
    bf16 = mybir.dt.bfloat16
    AF = mybir.ActivationFunctionType

    nc = bacc.Bacc("TRN2", target_bir_lowering=False, debug=False, num_devices=8)

    xq32 = nc.dram_tensor("xq32", [D, NQ], f32, kind="ExternalInput").ap()
    xtb = nc.dram_tensor("xtb", [D, S], bf16, kind="ExternalInput").ap()
    wqkv = nc.dram_tensor("wqkv", [D, 3 * D], bf16, kind="ExternalInput").ap()
    wotd = nc.dram_tensor("wotd", [D, D], bf16, kind="ExternalInput").ap()
    mskd = nc.dram_tensor("mskd", [P, 2 * P], bf16, kind="ExternalInput").ap()
    bqd = nc.dram_tensor("bqd", [D], f32, kind="ExternalInput").ap()
    bkd = nc.dram_tensor("bkd", [D], f32, kind="ExternalInput").ap()
    bod = nc.dram_tensor("bod", [D], f32, kind="ExternalInput").ap()
    gamd = nc.dram_tensor("gamd", [D], f32, kind="ExternalInput").ap()
    betd = nc.dram_tensor("betd", [D], f32, kind="ExternalInput").ap()
    yt = nc.dram_tensor("yt", [D, NQ], f32, kind="ExternalOutput").ap()

    xt_r = xtb.rearrange("(dk p) t -> p dk t", p=P)
    xq32_r = xq32.rearrange("(ok p) q -> p ok q", p=P)
    w_r = wqkv.rearrange("(dk p) (s c) -> p dk s c", p=P, c=P)  # 24 sections
    wot_r = wotd.rearrange("(dk p) (o c) -> p dk o c", p=P, c=P)

    with tile.TileContext(nc) as tc:
        with tc.tile_pool(name="pers", bufs=1) as pers:
            msk = pers.tile([P, 2, P], bf16)
            bias = pers.tile([P, DK, 5], f32)      # bq bk bo' gam bet
            ones = pers.tile([P, 1], f32r)
            eps = pers.tile([1, 1], f32)

            def bq_(f): return bias[:, f, 0:1]
            def bk_(f): return bias[:, f, 1:2]
            def bo_(f): return bias[:, f, 2:3]
            def gam_(f): return bias[:, f, 3:4]
            def bet_(f): return bias[:, f, 4:5]

            def body(g):
                qss, nks = QSTART[g], NKS[g]
                with tc.tile_pool(name="big", bufs=1) as big:
                    kt = big.tile([P, DK, S], bf16, name=f"kt{g}")
                    v = big.tile([P, NKT, H, HD + 1], bf16, name=f"v{g}")
                    qa = big.tile([P, DK, NQ], bf16, name=f"qa{g}")
                    ctxt0 = big.tile([P, DK, QT], bf16, name=f"ctxt{g}_0")
                    ctxt1 = big.tile([P, DK, QT], bf16, name=f"ctxt{g}_1")
                    y = big.tile([P, DK, QT], f32r, name=f"y{g}")
                    nc.vector.memset(v[:, :, :, HD], 1.0)
                    nc.vector.memset(eps[:], 1e-5)
                    nc.vector.memset(ones[:].bitcast(f32), 1.0 / D)

                    # ---- projections: K, V, Q -------------------------
                    with (
                        tc.tile_pool(name="xtp", bufs=1) as xtp,
                        tc.tile_pool(name="wsp", bufs=2) as wsp,
                        tc.tile_pool(name="pp8", bufs=4, space="PSUM") as pp8,
                    ):
                        xt = xtp.tile([P, DK, S], bf16, name=f"xt{g}")
                        # K^T: stationary w chunk reused over 4 t-tiles
                        wk = wsp.tile([P, DK, 8, P], bf16, tag="w", name=f"wk{g}")
                        nc.gpsimd.dma_start(wk[:], w_r[:, :, 8:16, :])
                        for t in range(4):
                            nc.sync.dma_start(
                                xt[:, :, ts(t, QT)], xt_r[:, :, ts(t, QT)])
                        for j, bsrc in enumerate((bqd, bkd, bod, gamd, betd)):
                            nc.sync.dma_start(
                                bias[:, :, j], bsrc.rearrange("(f p) -> p f", p=P))
                        nc.sync.dma_start(
                            msk[:], mskd.rearrange("p (h c) -> p h c", c=P))
                        for f in range(DK):
                            pts = [
                                pp8.tile([P, QT], f32, tag="pp", name=f"pk{g}_{f}_{t}")
                                for t in range(4)
                            ]
                            for dk in range(DK):
                                for t in range(4):
                                    nc.tensor.matmul(
                                        pts[t][:], wk[:, dk, f, :],
                                        xt[:, dk, ts(t, QT)],
                                        start=(dk == 0), stop=(dk == DK - 1),
                                    )
                            for t in range(4):
                                nc.scalar.add(kt[:, f, ts(t, QT)], pts[t][:], bk_(f))

                        # V natural: stationary x chunk reused over 2 f-groups
                        wv = wsp.tile([P, DK, 8, P], bf16, tag="w", name=f"wv{g}")
                        nc.gpsimd.dma_start(wv[:], w_r[:, :, 16:24, :])
                        for t in range(NKT):
                            pv = [
                                pp8.tile([P, 8, HD], f32, tag="pp", name=f"pv{g}_{t}_{fg}")
                                for fg in range(2)
                            ]
                            for dk in range(DK):
                                for fg in range(2):
                                    nc.tensor.matmul(
                                        pv[fg][:], xt[:, dk, ts(t, P)],
                                        wv[:, dk, 4 * fg:4 * (fg + 1), :],
                                        start=(dk == 0), stop=(dk == DK - 1),
                                    )
                            for fg in range(2):
                                nc.scalar.copy(
                                    v[:, t, 8 * fg:8 * (fg + 1), 0:HD], pv[fg][:])

                        # Q^T: stationary w chunk reused over both q-tiles
                        wq = wsp.tile([P, DK, 8, P], bf16, tag="w", name=f"wq{g}")
                        nc.gpsimd.dma_start(wq[:], w_r[:, :, 0:8, :])
                        for f in range(DK):
                            pq = [
                                pp8.tile([P, QT], f32, tag="pp", name=f"pq{g}_{f}_{qt}")
                                for qt in range(2)
                            ]
                            for dk in range(DK):
                                for qt in range(2):
                                    nc.tensor.matmul(
                                        pq[qt][:], wq[:, dk, f, :],
                                        xt[:, dk, qss[qt]:qss[qt] + QT],
                                        start=(dk == 0), stop=(dk == DK - 1),
                                    )
                            for qt in range(2):
                                nc.scalar.add(qa[:, f, ts(qt, QT)], pq[qt][:], bq_(f))

                    # ---- attention + out-proj + LN --------------------
                    with (
                        tc.tile_pool(name="sps", bufs=2, space="PSUM") as sps,
                        tc.tile_pool(name="cps", bufs=3, space="PSUM") as cps,
                        tc.tile_pool(name="pp2", bufs=1, space="PSUM") as pp2,
                        tc.tile_pool(name="sep", bufs=3) as sep,
                        tc.tile_pool(name="wop", bufs=2) as wop,
                        tc.tile_pool(name="ep", bufs=2) as ep,
                    ):
                        for qt in range(2):
                            nk = nks[qt]
                            qb = qt * QT
                            ctx_t = ctxt0 if qt == 0 else ctxt1
                            # ---- attention ----
                            for hp in range(H // 2):
                                cp0 = cps.tile([HD + 1, QT], f32, tag="cp",
                                               name=f"cp0_{g}_{qt}_{hp}")
                                cp1 = cps.tile([HD + 1, QT], f32, tag="cp",
                                               name=f"cp1_{g}_{qt}_{hp}")
                                for i in range(nk):
                                    bi = i - (nk - 4)
                                    off = P * bi if bi > 0 else 0
                                    sp = sps.tile([P, 2, QT], f32, tag="sp",
                                                  name=f"sp{g}_{qt}_{hp}_{i}")
                                    nc.tensor.matmul(
                                        sp[:, 0, off:], kt[0:HD, hp, ts(i, P)],
                                        qa[0:HD, hp, qb + off:qb + QT],
                                        start=True, stop=True,
                                    )
                                    nc.tensor.matmul(
                                        sp[:, 1, off:], kt[HD:P, hp, ts(i, P)],
                                        qa[HD:P, hp, qb + off:qb + QT],
                                        start=True, stop=True,
                                    )
                                    se = sep.tile([P, 2, QT], bf16, tag="se",
                                                  name=f"se{g}_{qt}_{hp}_{i}")
                                    nc.scalar.activation(
                                        se[:, :, off:], sp[:, :, off:],
                                        AF.Exp, scale=0.125)
                                    if bi >= 0:
                                        nc.vector.tensor_mul(
                                            se[:, :, off:off + P],
                                            se[:, :, off:off + P], msk[:])
                                    nc.tensor.matmul(
                                        cp0[:, off:], v[:, i, 2 * hp, :],
                                        se[:, 0, off:],
                                        start=(i == 0), stop=(i == nk - 1),
                                    )
                                    nc.tensor.matmul(
                                        cp1[:, off:], v[:, i, 2 * hp + 1, :],
                                        se[:, 1, off:],
                                        start=(i == 0), stop=(i == nk - 1),
                                    )
                                den = ep.tile([1, 2, QT], f32, tag="den", bufs=1,
                                              name=f"den{g}_{qt}_{hp}")
                                dcp = nc.scalar.copy if qt == 0 else (
                                    lambda o_, i_: nc.vector.tensor_copy(o_, i_))
                                dcp(den[:, 0, :], cp0[HD:HD + 1, :])
                                dcp(den[:, 1, :], cp1[HD:HD + 1, :])
                                rden = ep.tile([1, 2 * QT], f32, tag="rden",
                                               name=f"rden{g}_{qt}_{hp}")
                                for j in range(2):
                                    nc.vector.reciprocal_approx_fast(
                                        rden[:, ts(j, QT)], den[:, j, :])
                                bch = ep.tile([HD, 2 * QT], f32, tag="bch",
                                              name=f"bch{g}_{qt}_{hp}")
                                nc.gpsimd.partition_broadcast(bch[:], rden[:])
                                nc.vector.tensor_mul(
                                    ctx_t[0:HD, hp, :], cp0[0:HD, :],
                                    bch[:, ts(0, QT)])
                                nc.vector.tensor_mul(
                                    ctx_t[HD:P, hp, :], cp1[0:HD, :],
                                    bch[:, ts(1, QT)])

                        for qt in range(2):
                            ctx_t = ctxt0 if qt == 0 else ctxt1
                            # ---- out proj + residual + LN ----
                            for o in range(DK):
                                wo = wop.tile([P, DK, P], bf16, tag="wo",
                                              name=f"wo{g}_{qt}_{o}")
                                nc.gpsimd.dma_start(wo[:], wot_r[:, :, o, :])
                                ps = pp2.tile([P, QT], f32, tag="pp2",
                                              name=f"po{g}_{qt}_{o}")
                                for dk in range(DK):
                                    nc.tensor.matmul(
                                        ps[:], wo[:, dk, :], ctx_t[:, dk, :],
                                        start=(dk == 0), stop=(dk == DK - 1),
                                    )
                                xr = ep.tile([P, QT], f32, tag="xr", bufs=3,
                                             name=f"xr{g}_{qt}_{o}")
                                nc.sync.dma_start(xr[:], xq32_r[:, o, ts(qt, QT)])
                                nc.vector.scalar_tensor_tensor(
                                    y[:, o, :], ps[:], bo_(o), xr[:],
                                    AluOpType.add, AluOpType.add,
                                )
                            mu_ps = pp2.tile([P, QT], f32, tag="pp2",
                                             name=f"mups{g}_{qt}")
                            for o in range(DK):
                                nc.tensor.matmul(
                                    mu_ps[0:1, :], ones[:], y[:, o, :],
                                    start=(o == 0), stop=(o == DK - 1))
                            ysq = ep.tile([P, DK, QT], f32r, tag="ynq", bufs=1,
                                          name=f"ysq{g}_{qt}")
                            ms_ps = pp2.tile([P, QT], f32, tag="pp2",
                                             name=f"msps{g}_{qt}")
                            for o in range(DK):
                                if qt == 0:
                                    nc.vector.tensor_mul(
                                        ysq[:, o, :], y[:, o, :], y[:, o, :])
                                else:
                                    nc.scalar.square(
                                        ysq[:, o, :], y[:, o, :].bitcast(f32))
                                nc.tensor.matmul(
                                    ms_ps[0:1, :], ones[:], ysq[:, o, :],
                                    start=(o == 0), stop=(o == DK - 1))
                            mu = ep.tile([1, QT], f32, tag="mu", bufs=1, name=f"mu{g}_{qt}")
                            nc.vector.tensor_copy(mu[:], mu_ps[0:1, :])
                            musq = ep.tile([1, QT], f32, tag="musq", bufs=1,
                                           name=f"musq{g}_{qt}")
                            nc.vector.tensor_mul(musq[:], mu[:], mu[:])
                            var = ep.tile([1, QT], f32, tag="var", bufs=1, name=f"var{g}_{qt}")
                            nc.vector.tensor_sub(var[:], ms_ps[0:1, :], musq[:])
                            lnv = ep.tile([1, QT], f32, tag="lnv", bufs=1, name=f"lnv{g}_{qt}")
                            nc.scalar.activation(lnv[:], var[:], AF.Ln, bias=eps[:])
                            rstd = ep.tile([1, QT], f32, tag="rstd",
                                           name=f"rstd{g}_{qt}")
                            nc.scalar.activation(rstd[:], lnv[:], AF.Exp, scale=-0.5)
                            mu_bc = ep.tile([P, QT], f32, tag="mu_bc", bufs=1,
                                            name=f"mubc{g}_{qt}")
                            nc.gpsimd.partition_broadcast(mu_bc[:], mu[:])
                            rs_bc = ep.tile([P, QT], f32, tag="rs_bc", bufs=1,
                                            name=f"rsbc{g}_{qt}")
                            nc.gpsimd.partition_broadcast(rs_bc[:], rstd[:])
                            mu_b = mu_bc[:].unsqueeze(1).broadcast_to([P, DK, QT])
                            rs_b = rs_bc[:].unsqueeze(1).broadcast_to([P, DK, QT])
                            yn = ep.tile([P, DK, QT], f32, tag="ynq", bufs=1,
                                         name=f"yn{g}_{qt}")
                            nc.vector.tensor_sub(yn[:], y[:].bitcast(f32), mu_b)
                            nc.vector.tensor_mul(yn[:], yn[:], rs_b)
                            for o in range(DK):
                                yo = ep.tile([P, QT], f32, tag="yo",
                                             name=f"yo{g}_{qt}_{o}")
                                nc.vector.tensor_scalar(
                                    yo[:], yn[:, o, :], gam_(o), bet_(o),
                                    AluOpType.mult, AluOpType.add,
                                )
                                nc.sync.dma_start(yt[ts(o, P), ts(qt, QT)], yo[:])

            pid = nc.partition_id()
            with tc.If(pid < 4) as cmp:
                body(0)
            with cmp.Else():
                body(1)
    nc.compile()
    return nc


def _get_nc():
    if "nc" not in _cache:
        _cache["nc"] = _build()
    return _cache["nc"]


def _prep(x, in_proj_w, in_proj_b, out_w, out_b, gamma, beta):
    import ml_dtypes
    bf16 = ml_dtypes.bfloat16

    x = np.asarray(x, np.float32)
    w = np.asarray(in_proj_w, np.float32)
    wt = np.ascontiguousarray(w.T)                          # [D, 3D]
    wo = np.asarray(out_w, np.float32)
    wot = np.ascontiguousarray(wo.T)                        # [D, D]
    bqkv = np.asarray(in_proj_b, np.float32)
    bo2 = np.asarray(out_b, np.float32) + wo @ bqkv[2 * D:3 * D]
    gam = np.asarray(gamma, np.float32)
    bet = np.asarray(beta, np.float32)
    tri = (np.arange(P)[:, None] <= np.arange(P)[None, :])
    mskd = np.ascontiguousarray(
        np.concatenate([tri, tri], axis=1).astype(bf16))
    wqkv_b = wt.astype(bf16)
    wot_b = wot.astype(bf16)
    qcols = {
        0: np.r_[0:QT, 3 * QT:4 * QT],
        1: np.r_[QT:3 * QT],
    }
    in_maps = []
    for c in range(8):
        b, g = c % 4, c // 4
        xt32 = np.ascontiguousarray(x[b].T)
        in_maps.append({
            "xtb": xt32.astype(bf16),
            "xq32": np.ascontiguousarray(xt32[:, qcols[g]]),
            "wqkv": wqkv_b,
            "wotd": wot_b,
            "mskd": mskd,
            "bqd": bqkv[0:D], "bkd": bqkv[D:2 * D], "bod": bo2,
            "gamd": gam, "betd": bet,
        })
    return in_maps, qcols


def _run(in_maps, trace=False, **kw):
    from concourse.bass_utils import run_bass_kernel_spmd

    return run_bass_kernel_spmd(_get_nc(), in_maps, list(range(8)), trace=trace, **kw)


def kernel(x, in_proj_w, in_proj_b, out_w, out_b, gamma, beta):
    in_maps, qcols = _prep(x, in_proj_w, in_proj_b, out_w, out_b, gamma, beta)
    res = _run(in_maps)
    out = np.empty((B, S, D), np.float32)
    for c in range(8):
        out[c % 4, qcols[c // 4]] = res.results[c]["yt"].T
    return out
